# revision 17
# baseline (speedup 1.0000x reference)
"""Causal self-attention on 8 NeuronCores (Trainium2, Bass/Tile).

Sharding: core c handles batch b = c//2 and head-group hg = c%2
(8 of 16 heads = 512 of 1024 feature dims). W_qkv is split column-wise,
W_proj row-wise per head group; each core returns a partial [T, D]
projection output and the host sums the two partials per batch.

Per-core dataflow (all matmuls fp32r except P/V which are bf16):
  xT = embds[b].T              [1024, 2048]  (host-transposed)
  qT/kT = Wq/Wk.T @ x.T        [512, 2048]   (head-dim major)
  v     = x @ Wv               [2048, 512]   (natural, + ones col per head)
  sT[j,i] = kT.T @ qT          per head, causal-skipped/shrunk tiles
  PT = exp(SCALE * sT) (*mask on diagonal strips)   bf16
  UT[e,i], denom[i] = [v|1].T @ PT                  (ones col -> denom)
  affinT = UT * (1/denom)      broadcast via K=8 matmul with E matrix
  partial = affinT.T @ Wp      accumulated over e-chunks, DMA'd out
"""

import sys

for _p in ("/opt/trn_rl_repo",):
    if _p not in sys.path:
        sys.path.append(_p)

import ml_dtypes
import numpy as np

import concourse.bass as bass
import concourse.tile as tile
from concourse import bacc, mybir
from concourse.bass_utils import run_bass_kernel_spmd

F32 = mybir.dt.float32
F32R = mybir.dt.float32r
BF16 = mybir.dt.bfloat16
EXP = mybir.ActivationFunctionType.Exp
COPY = mybir.ActivationFunctionType.Copy

B, T, D = 4, 2048, 1024
H, Dh = 16, 64
SCALE = float(D) ** -0.5
NCORES = 8
DL = 512          # local (per-core) feature width = 8 heads * 64
HL = 8            # local heads
NDC = D // 128    # 8 d-chunks
NEC = DL // 128   # 4 e-chunks (head pairs)
NTB = T // 512    # 4 t-blocks of 512
NTC = T // 128    # 16 t-chunks of 128
VPAIR = 192       # v_sb per-pair block: [v_even(64) | one | junk(63) | v_odd(64)]
VROW = NEC * VPAIR  # 640 cols per v_sb tile


def _build():
    nc = bacc.Bacc("TRN2", target_bir_lowering=False, debug=False,
                   num_devices=NCORES)

    xT = nc.declare_dram_parameter("xT", [D, T], F32R, isOutput=False)
    wq = nc.declare_dram_parameter("wq", [D, DL], F32R, isOutput=False)
    wk = nc.declare_dram_parameter("wk", [D, DL], F32R, isOutput=False)
    wv = nc.declare_dram_parameter("wv", [D, DL], F32R, isOutput=False)
    wp = nc.declare_dram_parameter("wp", [DL, D], BF16, isOutput=False)
    mask = nc.declare_dram_parameter("mask", [128, 128], BF16, isOutput=False)
    emat = nc.declare_dram_parameter("emat", [HL, DL], F32R, isOutput=False)
    out = nc.declare_dram_parameter("out", [T, D], F32, isOutput=True)

    with tile.TileContext(nc) as tc:
        _emit(nc, tc, xT, wq, wk, wv, wp, mask, emat, out)
    nc.compile()
    return nc


def _emit(nc, tc, xT, wq, wk, wv, wp, mask, emat, out):
    from contextlib import ExitStack

    ctx = ExitStack()
    with ctx:
        wqk_pool = ctx.enter_context(tc.tile_pool(name="wqk", bufs=16))
        qk_pool = ctx.enter_context(tc.tile_pool(name="qk", bufs=6))
        vsb_pool = ctx.enter_context(tc.tile_pool(name="vsb", bufs=NTC))
        ut_pool = ctx.enter_context(tc.tile_pool(name="ut", bufs=NEC))
        dn_pool = ctx.enter_context(tc.tile_pool(name="dn", bufs=1))
        dns_pool = ctx.enter_context(tc.tile_pool(name="dns", bufs=2))
        pt_pool = ctx.enter_context(tc.tile_pool(name="pt", bufs=3))
        cst_pool = ctx.enter_context(tc.tile_pool(name="cst", bufs=1))
        ps_pool = ctx.enter_context(tc.tile_pool(name="ps", bufs=1, space="PSUM"))

        # constants
        mk_sb = cst_pool.tile([128, 128], BF16, tag="mk")
        nc.sync.dma_start(mk_sb[:], mask[:])
        em_sb = cst_pool.tile([HL, DL], F32R, tag="em")
        nc.sync.dma_start(em_sb[:], emat[:])

        # persistent tiles
        ut_sb = [ut_pool.tile([128, T], BF16, tag="ut", name=f"ut{i}")
                 for i in range(NEC)]
        dn_sb = dn_pool.tile([HL, T], F32, tag="dn")
        rd_sb = dn_pool.tile([HL, T], F32R, tag="rd")
        v_sb = [vsb_pool.tile([128, VROW], BF16, tag="vsb", name=f"vsb{i}")
                for i in range(NTC)]
        # garbage rows of dn would hit reciprocal before they are written;
        # keep them finite so 0*inf NaNs can't leak out of the R matmul
        nc.gpsimd.memset(dn_sb[:], 1.0)

        def ps_tile(tag, bufs):
            return ps_pool.tile([128, 512], F32, tag=tag, name=f"ps_{tag}",
                                bufs=bufs)

        with tc.tile_pool(name="xt", bufs=NDC) as xt_pool:
            xt_sb = []
            qk_chunks = {}

            def qk_filler(ec):
                """Generator computing q/k chunks for `ec`; yields between
                small PE steps so it can be dripped into the attention loop
                as filler work that keeps the PE dense (HAM stays warm).
                kT is stored twice, zero-padded per head parity, so the
                score matmuls run with K=128 (K=64 fp32r matmuls throttle
                to half rate when ACT runs concurrently)."""
                wq_t, wk_t = [], []
                for dc in range(NDC):
                    t = wqk_pool.tile([128, 128], F32R, tag="wqk", name="wqkt")
                    nc.sync.dma_start(
                        t[:], wq[dc * 128:(dc + 1) * 128,
                                 ec * 128:(ec + 1) * 128])
                    wq_t.append(t)
                for dc in range(NDC):
                    t = wqk_pool.tile([128, 128], F32R, tag="wqk", name="wqkt")
                    nc.sync.dma_start(
                        t[:], wk[dc * 128:(dc + 1) * 128,
                                 ec * 128:(ec + 1) * 128])
                    wk_t.append(t)
                q_ec = qk_pool.tile([128, T], F32R, tag="qk", name="q_ec")
                kA = qk_pool.tile([128, T], F32R, tag="qk", name="kA")
                kB = qk_pool.tile([128, T], F32R, tag="qk", name="kB")
                nc.vector.memset(kA[64:128, :].bitcast(F32), 0.0)
                nc.vector.memset(kB[0:64, :].bitcast(F32), 0.0)
                qk_chunks[ec] = (q_ec, kA, kB)
                for (w_t, iskA) in ((wq_t, False), (wk_t, True)):
                    for tbp in range(2):
                        pss = [ps_tile("qkps", 2) for _ in range(2)]
                        for dc in range(NDC):
                            for i in range(2):
                                tb = 2 * tbp + i
                                nc.tensor.matmul(
                                    pss[i], w_t[dc][:],
                                    xt_sb[dc][:, tb * 512:(tb + 1) * 512],
                                    start=(dc == 0), stop=(dc == NDC - 1))
                            yield
                        for i in range(2):
                            tb = 2 * tbp + i
                            sl = slice(tb * 512, (tb + 1) * 512)
                            if iskA:
                                nc.vector.tensor_copy(
                                    kA[0:64, sl], pss[i][0:64, :])
                                nc.vector.tensor_copy(
                                    kB[64:128, sl], pss[i][64:128, :])
                            else:
                                nc.vector.tensor_copy(q_ec[:, sl], pss[i][:])
                        yield

            def normalize(ec):
                """affinT = UT * 1/denom for chunk ec (dripped into the next
                chunk's attention so nothing stalls on it)."""
                with nc.allow_low_precision(reason="f32r denom reciprocal"):
                    nc.vector.reciprocal(rd_sb[:], dn_sb[:])
                for tb in range(NTB):
                    ps_r = ps_tile("qkps", 2)
                    nc.tensor.matmul(
                        ps_r[:], em_sb[:, ec * 128:(ec + 1) * 128],
                        rd_sb[:, tb * 512:(tb + 1) * 512],
                        start=True, stop=True)
                    nc.vector.tensor_mul(
                        ut_sb[ec][:, tb * 512:(tb + 1) * 512],
                        ut_sb[ec][:, tb * 512:(tb + 1) * 512], ps_r[:])

            # ------------- phase A0: v = x @ Wv (+ dripped qk(0)) -------------
            with tc.tile_pool(name="wv", bufs=NDC) as wv_pool:
                wv_sb = []
                for dc in range(NDC):
                    t = wv_pool.tile([128, DL], F32R, tag="wv", name=f"wv{dc}")
                    nc.sync.dma_start(t[:], wv[dc * 128:(dc + 1) * 128, :])
                    wv_sb.append(t)
                for dc in range(NDC):
                    t = xt_pool.tile([128, T], F32R, tag="xt", name=f"xt{dc}")
                    nc.sync.dma_start(t[:], xT[dc * 128:(dc + 1) * 128, :])
                    xt_sb.append(t)

                filler0 = qk_filler(0)
                for tcn in range(NTC):
                    ps_v = ps_tile("utps", 2)
                    for dc in range(NDC):
                        nc.tensor.matmul(
                            ps_v[:], xt_sb[dc][:, tcn * 128:(tcn + 1) * 128],
                            wv_sb[dc][:], start=(dc == 0),
                            stop=(dc == NDC - 1))
                    dst = v_sb[tcn][:].rearrange("p (e c) -> p e c", c=VPAIR)
                    src = ps_v[:].rearrange("p (e c) -> p e c", c=128)
                    nc.vector.tensor_copy(dst[:, :, 0:64], src[:, :, 0:64])
                    nc.vector.tensor_copy(dst[:, :, 128:192], src[:, :, 64:128])
                    nc.gpsimd.memset(dst[:, :, 64:65], 1.0)
                    nc.gpsimd.memset(dst[:, :, 65:128], 0.0)
                    next(filler0, None)
                for _ in filler0:
                    pass
            # wv pool released here

            # ------------- per e-chunk: attention + dripped qk(ec+1) -------------
            for ec in range(NEC):
                filler = qk_filler(ec + 1) if ec + 1 < NEC else iter(())
                q_ec, kA, kB = qk_chunks.pop(ec)
                for par in range(2):       # head parity within chunk
                    h = 2 * ec + par       # local head index
                    kpad = kA if par == 0 else kB
                    for ibp in range(2):   # i-block pair (2*ibp, 2*ibp+1)
                        ibl, ibr = 2 * ibp, 2 * ibp + 1
                        utl = ps_tile("utps", 2)
                        utr = ps_tile("utps", 2)
                        for jt in range(4 * ibr + 4):
                            next(filler, None)
                            if ec > 0 and par == 0 and ibp == 0 and jt == 6:
                                normalize(ec - 1)
                            dl = (jt // 4 == ibl)
                            skip_l = (jt // 4 > ibl)
                            dr = (jt // 4 == ibr)
                            cl = 128 * (jt - 4 * ibl) if dl else 0
                            cr = 128 * (jt - 4 * ibr) if dr else 0
                            c0 = 512 + cr if skip_l else cl
                            st_ps = ps_pool.tile([128, 1024], F32, tag="stps",
                                                 name="ps_stps", bufs=2)
                            kh_j = kpad[:, jt * 128:(jt + 1) * 128]
                            if not skip_l:
                                nc.tensor.matmul(
                                    st_ps[:, cl:512], kh_j,
                                    q_ec[:, ibl * 512 + cl:(ibl + 1) * 512],
                                    start=True, stop=True)
                            nc.tensor.matmul(
                                st_ps[:, 512 + cr:1024], kh_j,
                                q_ec[:, ibr * 512 + cr:(ibr + 1) * 512],
                                start=True, stop=True)
                            pt_t = pt_pool.tile([128, 1024], BF16, tag="pt")
                            nc.scalar.activation(
                                pt_t[:, c0:1024], st_ps[:, c0:1024], EXP,
                                scale=SCALE)
                            if dl:
                                nc.vector.tensor_mul(
                                    pt_t[:, cl:cl + 128],
                                    pt_t[:, cl:cl + 128], mk_sb[:])
                            if dr:
                                nc.vector.tensor_mul(
                                    pt_t[:, 512 + cr:512 + cr + 128],
                                    pt_t[:, 512 + cr:512 + cr + 128], mk_sb[:])
                            # PV: [v|1].T @ PT -> UT rows + denom row
                            vt = v_sb[jt][:].rearrange(
                                "p (e c) -> p e c", c=VPAIR)[:, ec, :]
                            if par == 0:
                                lhs = vt[:, 0:65]       # M=65 -> rows 0..64
                                rsl = slice(0, 65)
                            else:
                                # [one|junk63|v_odd]: denom row 0, v 64..127
                                lhs = vt[:, 64:192]     # M=128
                                rsl = slice(0, 128)
                            if not skip_l:
                                nc.tensor.matmul(
                                    utl[rsl, cl:512], lhs, pt_t[:, cl:512],
                                    start=(jt == 0), stop=(jt == 4 * ibl + 3),
                                    skip_group_check=True)
                            nc.tensor.matmul(
                                utr[rsl, cr:512], lhs, pt_t[:, 512 + cr:1024],
                                start=(jt == 0), stop=(jt == 4 * ibr + 3),
                                skip_group_check=True)
                            for ib_d, ut_d in ((ibl, utl), (ibr, utr)):
                                if jt != 4 * ib_d + 3:
                                    continue
                                if par == 0:
                                    usrc, dsrc, r = (ut_d[0:64, :],
                                                     ut_d[64:65, :], 64)
                                    udst = ut_sb[ec][
                                        0:64, ib_d * 512:(ib_d + 1) * 512]
                                else:
                                    usrc, dsrc, r = (ut_d[64:128, :],
                                                     ut_d[0:1, :], 0)
                                    udst = ut_sb[ec][
                                        64:128, ib_d * 512:(ib_d + 1) * 512]
                                nc.scalar.copy(udst, usrc)
                                # denom: same-partition copy, then DMA repack
                                stg = dns_pool.tile([128, 512], F32,
                                                    tag="dns", name="dnstg")
                                nc.vector.tensor_copy(stg[r:r + 1, :], dsrc)
                                nc.sync.dma_start(
                                    dn_sb[h:h + 1,
                                          ib_d * 512:(ib_d + 1) * 512],
                                    stg[r:r + 1, :])
                for _ in filler:   # drain remaining qk(ec+1) work
                    pass
            normalize(NEC - 1)
        # xt pool released here

        # ---------------- projection: out = affinT.T @ wp (bf16) ----------------
        with tc.tile_pool(name="wp", bufs=NEC) as wp_pool, \
             tc.tile_pool(name="stage", bufs=2) as stage_pool:
            wp_sb = []
            for ecn in range(NEC):
                t = wp_pool.tile([128, D], BF16, tag="wp", name=f"wpt{ecn}")
                nc.sync.dma_start(t[:], wp[ecn * 128:(ecn + 1) * 128, :])
                wp_sb.append(t)
            for tcn in range(NTC):
                for ob in range(2):
                    ps_p = ps_tile("qkps", 2)
                    for ecn in range(NEC):
                        nc.tensor.matmul(
                            ps_p[:], ut_sb[ecn][:, tcn * 128:(tcn + 1) * 128],
                            wp_sb[ecn][:, ob * 512:(ob + 1) * 512],
                            start=(ecn == 0), stop=(ecn == NEC - 1))
                    st = stage_pool.tile([128, 512], F32, tag="st", name="stg")
                    nc.scalar.copy(st[:], ps_p[:])
                    nc.sync.dma_start(
                        out[tcn * 128:(tcn + 1) * 128,
                            ob * 512:(ob + 1) * 512], st[:])


_NC_CACHE = None


def _get_nc():
    global _NC_CACHE
    if _NC_CACHE is None:
        _NC_CACHE = _build()
    return _NC_CACHE


def make_in_maps(embds, W_qkv, W_proj):
    embds = np.asarray(embds, dtype=np.float32)
    W_qkv = np.asarray(W_qkv, dtype=np.float32)
    W_proj = np.asarray(W_proj, dtype=np.float32)

    mask_np = np.triu(np.ones((128, 128))).astype(ml_dtypes.bfloat16)
    emat_np = np.kron(np.eye(HL), np.ones((1, Dh))).astype(np.float32)

    in_maps = []
    for c in range(NCORES):
        b, hg = c // 2, c % 2
        sl = slice(hg * DL, (hg + 1) * DL)
        in_maps.append({
            "xT": np.ascontiguousarray(embds[b].T),
            "wk": np.ascontiguousarray(W_qkv[:, 0 * D:1 * D][:, sl]),
            "wq": np.ascontiguousarray(W_qkv[:, 1 * D:2 * D][:, sl]),
            "wv": np.ascontiguousarray(W_qkv[:, 2 * D:3 * D][:, sl]),
            "wp": np.ascontiguousarray(W_proj[sl, :]).astype(ml_dtypes.bfloat16),
            "mask": mask_np,
            "emat": emat_np,
        })
    return in_maps


def gather_out(outs, b_proj):
    b_proj = np.asarray(b_proj, dtype=np.float32)
    full = np.empty((B, T, D), dtype=np.float32)
    for b in range(B):
        full[b] = outs[2 * b] + outs[2 * b + 1] + b_proj[None, :]
    return full


def kernel(embds, W_qkv, W_proj, b_proj):
    in_maps = make_in_maps(embds, W_qkv, W_proj)
    nc = _get_nc()
    res = run_bass_kernel_spmd(nc, in_maps, list(range(NCORES)))
    return gather_out([r["out"] for r in res.results], b_proj)


# revision 19
# speedup vs baseline: 1.0868x; 1.0868x over previous
"""Causal self-attention on 8 NeuronCores (Trainium2, Bass/Tile).

Sharding: core c handles batch b = c//2 and head-group hg = c%2
(8 of 16 heads = 512 of 1024 feature dims). W_qkv is split column-wise,
W_proj row-wise per head group; each core returns a partial [T, D]
projection output and the host sums the two partials per batch.

Per-core dataflow (all matmuls fp32r except P/V which are bf16):
  xT = embds[b].T              [1024, 2048]  (host-transposed)
  qT/kT = Wq/Wk.T @ x.T        [512, 2048]   (head-dim major)
  v     = x @ Wv               [2048, 512]   (natural, + ones col per head)
  sT[j,i] = kT.T @ qT          per head, causal-skipped/shrunk tiles
  PT = exp(SCALE * sT) (*mask on diagonal strips)   bf16
  UT[e,i], denom[i] = [v|1].T @ PT                  (ones col -> denom)
  affinT = UT * (1/denom)      broadcast via K=8 matmul with E matrix
  partial = affinT.T @ Wp      accumulated over e-chunks, DMA'd out
"""

import sys

for _p in ("/opt/trn_rl_repo",):
    if _p not in sys.path:
        sys.path.append(_p)

import ml_dtypes
import numpy as np

import concourse.bass as bass
import concourse.tile as tile
from concourse import bacc, mybir
from concourse.bass_utils import run_bass_kernel_spmd

F32 = mybir.dt.float32
F32R = mybir.dt.float32r
BF16 = mybir.dt.bfloat16
EXP = mybir.ActivationFunctionType.Exp
COPY = mybir.ActivationFunctionType.Copy

B, T, D = 4, 2048, 1024
H, Dh = 16, 64
SCALE = float(D) ** -0.5
NCORES = 8
DL = 512          # local (per-core) feature width = 8 heads * 64
HL = 8            # local heads
NDC = D // 128    # 8 d-chunks
NEC = DL // 128   # 4 e-chunks (head pairs)
NTB = T // 512    # 4 t-blocks of 512
NTC = T // 128    # 16 t-chunks of 128
VPAIR = 192       # v_sb per-pair block: [v_even(64) | one | junk(63) | v_odd(64)]
VROW = NEC * VPAIR  # 640 cols per v_sb tile


def _build():
    nc = bacc.Bacc("TRN2", target_bir_lowering=False, debug=False,
                   num_devices=NCORES)

    xT = nc.declare_dram_parameter("xT", [D, T], F32R, isOutput=False)
    wq = nc.declare_dram_parameter("wq", [D, DL], F32R, isOutput=False)
    wk = nc.declare_dram_parameter("wk", [D, DL], F32R, isOutput=False)
    wv = nc.declare_dram_parameter("wv", [D, DL], F32R, isOutput=False)
    wp = nc.declare_dram_parameter("wp", [DL, D], BF16, isOutput=False)
    mask = nc.declare_dram_parameter("mask", [128, 128], BF16, isOutput=False)
    emat = nc.declare_dram_parameter("emat", [HL, DL], F32, isOutput=False)
    out = nc.declare_dram_parameter("out", [T, D], F32, isOutput=True)

    with tile.TileContext(nc) as tc:
        _emit(nc, tc, xT, wq, wk, wv, wp, mask, emat, out)
    nc.compile()
    return nc


def _emit(nc, tc, xT, wq, wk, wv, wp, mask, emat, out):
    from contextlib import ExitStack

    ctx = ExitStack()
    with ctx:
        wqk_pool = ctx.enter_context(tc.tile_pool(name="wqk", bufs=16))
        qk_pool = ctx.enter_context(tc.tile_pool(name="qk", bufs=6))
        vsb_pool = ctx.enter_context(tc.tile_pool(name="vsb", bufs=NTC))
        ut_pool = ctx.enter_context(tc.tile_pool(name="ut", bufs=NEC))
        dn_pool = ctx.enter_context(tc.tile_pool(name="dn", bufs=1))
        dns_pool = ctx.enter_context(tc.tile_pool(name="dns", bufs=2))
        pt_pool = ctx.enter_context(tc.tile_pool(name="pt", bufs=3))
        cst_pool = ctx.enter_context(tc.tile_pool(name="cst", bufs=1))
        ps_pool = ctx.enter_context(tc.tile_pool(name="ps", bufs=1, space="PSUM"))

        # constants
        mk_sb = cst_pool.tile([128, 128], BF16, tag="mk")
        nc.sync.dma_start(mk_sb[:], mask[:])
        em_sb = cst_pool.tile([HL, DL], F32, tag="em")
        nc.sync.dma_start(em_sb[:], emat[:])

        # persistent tiles
        ut_sb = [ut_pool.tile([128, T], BF16, tag="ut", name=f"ut{i}")
                 for i in range(NEC)]
        dn_sb = dn_pool.tile([HL, T], F32, tag="dn")
        rd_sb = dn_pool.tile([HL, T], F32, tag="rd")
        v_sb = [vsb_pool.tile([128, VROW], BF16, tag="vsb", name=f"vsb{i}")
                for i in range(NTC)]
        # garbage rows of dn would hit reciprocal before they are written;
        # keep them finite so 0*inf NaNs can't leak out of the R matmul
        nc.gpsimd.memset(dn_sb[:], 1.0)

        def ps_tile(tag, bufs):
            return ps_pool.tile([128, 512], F32, tag=tag, name=f"ps_{tag}",
                                bufs=bufs)

        with tc.tile_pool(name="xt", bufs=NDC) as xt_pool:
            xt_sb = []
            qk_chunks = {}

            def qk_filler(ec):
                """Generator computing q/k chunks for `ec`; yields between
                small PE steps so it can be dripped into the attention loop
                as filler work that keeps the PE dense (HAM stays warm).
                kT is stored twice, zero-padded per head parity, so the
                score matmuls run with K=128 (K=64 fp32r matmuls throttle
                to half rate when ACT runs concurrently)."""
                wq_t, wk_t = [], []
                for dc in range(NDC):
                    t = wqk_pool.tile([128, 128], F32R, tag="wqk", name="wqkt")
                    nc.sync.dma_start(
                        t[:], wq[dc * 128:(dc + 1) * 128,
                                 ec * 128:(ec + 1) * 128])
                    wq_t.append(t)
                for dc in range(NDC):
                    t = wqk_pool.tile([128, 128], F32R, tag="wqk", name="wqkt")
                    nc.sync.dma_start(
                        t[:], wk[dc * 128:(dc + 1) * 128,
                                 ec * 128:(ec + 1) * 128])
                    wk_t.append(t)
                q_ec = qk_pool.tile([128, T], F32R, tag="qk", name="q_ec")
                kA = qk_pool.tile([128, T], F32R, tag="qk", name="kA")
                kB = qk_pool.tile([128, T], F32R, tag="qk", name="kB")
                nc.gpsimd.memset(kA[64:128, :].bitcast(F32), 0.0)
                nc.gpsimd.memset(kB[0:64, :].bitcast(F32), 0.0)
                qk_chunks[ec] = (q_ec, kA, kB)
                for (w_t, iskA) in ((wq_t, False), (wk_t, True)):
                    for tbp in range(2):
                        pss = [ps_tile("qkps", 2) for _ in range(2)]
                        for dc in range(NDC):
                            for i in range(2):
                                tb = 2 * tbp + i
                                nc.tensor.matmul(
                                    pss[i], w_t[dc][:],
                                    xt_sb[dc][:, tb * 512:(tb + 1) * 512],
                                    start=(dc == 0), stop=(dc == NDC - 1))
                            yield
                        for i in range(2):
                            tb = 2 * tbp + i
                            sl = slice(tb * 512, (tb + 1) * 512)
                            if iskA:
                                nc.vector.tensor_copy(
                                    kA[0:64, sl], pss[i][0:64, :])
                                nc.vector.tensor_copy(
                                    kB[64:128, sl], pss[i][64:128, :])
                            else:
                                nc.vector.tensor_copy(q_ec[:, sl], pss[i][:])
                        yield

            def normalize(ec):
                """affinT = UT * 1/denom for chunk ec (dripped into the next
                chunk's attention so nothing stalls on it)."""
                nc.vector.reciprocal_approx_fast(rd_sb[:], dn_sb[:])
                for tb in range(NTB):
                    ps_r = ps_tile("qkps", 2)
                    nc.tensor.matmul(
                        ps_r[:], em_sb[:, ec * 128:(ec + 1) * 128],
                        rd_sb[:, tb * 512:(tb + 1) * 512],
                        start=True, stop=True)
                    nc.vector.tensor_mul(
                        ut_sb[ec][:, tb * 512:(tb + 1) * 512],
                        ut_sb[ec][:, tb * 512:(tb + 1) * 512], ps_r[:])

            # ------------- phase A0: v = x @ Wv (+ dripped qk(0)) -------------
            with tc.tile_pool(name="wv", bufs=NDC) as wv_pool:
                wv_sb = []
                for dc in range(NDC):
                    t = wv_pool.tile([128, DL], F32R, tag="wv", name=f"wv{dc}")
                    nc.sync.dma_start(t[:], wv[dc * 128:(dc + 1) * 128, :])
                    wv_sb.append(t)
                    t = xt_pool.tile([128, T], F32R, tag="xt", name=f"xt{dc}")
                    nc.sync.dma_start(t[:], xT[dc * 128:(dc + 1) * 128, :])
                    xt_sb.append(t)

                filler0 = qk_filler(0)
                for tcn in range(NTC):
                    ps_v = ps_tile("utps", 2)
                    for dc in range(NDC):
                        nc.tensor.matmul(
                            ps_v[:], xt_sb[dc][:, tcn * 128:(tcn + 1) * 128],
                            wv_sb[dc][:], start=(dc == 0),
                            stop=(dc == NDC - 1))
                    dst = v_sb[tcn][:].rearrange("p (e c) -> p e c", c=VPAIR)
                    src = ps_v[:].rearrange("p (e c) -> p e c", c=128)
                    nc.vector.tensor_copy(dst[:, :, 0:64], src[:, :, 0:64])
                    nc.vector.tensor_copy(dst[:, :, 128:192], src[:, :, 64:128])
                    nc.gpsimd.memset(dst[:, :, 64:65], 1.0)
                    nc.gpsimd.memset(dst[:, :, 65:128], 0.0)
                    next(filler0, None)
                for _ in filler0:
                    pass
            # wv pool released here

            # ------------- per e-chunk: attention + dripped qk(ec+1) -------------
            for ec in range(NEC):
                filler = qk_filler(ec + 1) if ec + 1 < NEC else iter(())
                q_ec, kA, kB = qk_chunks.pop(ec)
                for par in range(2):       # head parity within chunk
                    h = 2 * ec + par       # local head index
                    kpad = kA if par == 0 else kB
                    for ibp in range(2):   # i-block pair (2*ibp, 2*ibp+1)
                        ibl, ibr = 2 * ibp, 2 * ibp + 1
                        utl = ps_tile("utps", 2)
                        utr = ps_tile("utps", 2)
                        for jt in range(4 * ibr + 4):
                            next(filler, None)
                            if ec > 0 and par == 0 and ibp == 0 and jt == 6:
                                normalize(ec - 1)
                            dl = (jt // 4 == ibl)
                            skip_l = (jt // 4 > ibl)
                            dr = (jt // 4 == ibr)
                            cl = 128 * (jt - 4 * ibl) if dl else 0
                            cr = 128 * (jt - 4 * ibr) if dr else 0
                            c0 = 512 + cr if skip_l else cl
                            st_ps = ps_pool.tile([128, 1024], F32, tag="stps",
                                                 name="ps_stps", bufs=2)
                            kh_j = kpad[:, jt * 128:(jt + 1) * 128]
                            if not skip_l:
                                nc.tensor.matmul(
                                    st_ps[:, cl:512], kh_j,
                                    q_ec[:, ibl * 512 + cl:(ibl + 1) * 512],
                                    start=True, stop=True)
                            nc.tensor.matmul(
                                st_ps[:, 512 + cr:1024], kh_j,
                                q_ec[:, ibr * 512 + cr:(ibr + 1) * 512],
                                start=True, stop=True)
                            pt_t = pt_pool.tile([128, 1024], BF16, tag="pt")
                            nc.scalar.activation(
                                pt_t[:, c0:1024], st_ps[:, c0:1024], EXP,
                                scale=SCALE)
                            if dl:
                                nc.vector.tensor_mul(
                                    pt_t[:, cl:cl + 128],
                                    pt_t[:, cl:cl + 128], mk_sb[:])
                            if dr:
                                nc.vector.tensor_mul(
                                    pt_t[:, 512 + cr:512 + cr + 128],
                                    pt_t[:, 512 + cr:512 + cr + 128], mk_sb[:])
                            # PV: [v|1].T @ PT -> UT rows + denom row
                            vt = v_sb[jt][:].rearrange(
                                "p (e c) -> p e c", c=VPAIR)[:, ec, :]
                            if par == 0:
                                lhs = vt[:, 0:65]       # M=65 -> rows 0..64
                                rsl = slice(0, 65)
                            else:
                                # [one|junk63|v_odd]: denom row 0, v 64..127
                                lhs = vt[:, 64:192]     # M=128
                                rsl = slice(0, 128)
                            if not skip_l:
                                nc.tensor.matmul(
                                    utl[rsl, cl:512], lhs, pt_t[:, cl:512],
                                    start=(jt == 0), stop=(jt == 4 * ibl + 3),
                                    skip_group_check=True)
                            nc.tensor.matmul(
                                utr[rsl, cr:512], lhs, pt_t[:, 512 + cr:1024],
                                start=(jt == 0), stop=(jt == 4 * ibr + 3),
                                skip_group_check=True)
                            for ib_d, ut_d in ((ibl, utl), (ibr, utr)):
                                if jt != 4 * ib_d + 3:
                                    continue
                                if par == 0:
                                    usrc, dsrc, r = (ut_d[0:64, :],
                                                     ut_d[64:65, :], 64)
                                    udst = ut_sb[ec][
                                        0:64, ib_d * 512:(ib_d + 1) * 512]
                                else:
                                    usrc, dsrc, r = (ut_d[64:128, :],
                                                     ut_d[0:1, :], 0)
                                    udst = ut_sb[ec][
                                        64:128, ib_d * 512:(ib_d + 1) * 512]
                                nc.scalar.copy(udst, usrc)
                                # denom: same-partition copy, then DMA repack
                                stg = dns_pool.tile([128, 512], F32,
                                                    tag="dns", name="dnstg")
                                nc.vector.tensor_copy(stg[r:r + 1, :], dsrc)
                                nc.sync.dma_start(
                                    dn_sb[h:h + 1,
                                          ib_d * 512:(ib_d + 1) * 512],
                                    stg[r:r + 1, :])
                for _ in filler:   # drain remaining qk(ec+1) work
                    pass
            normalize(NEC - 1)
        # xt pool released here

        # ---------------- projection: out = affinT.T @ wp (bf16) ----------------
        with tc.tile_pool(name="wp", bufs=NEC) as wp_pool, \
             tc.tile_pool(name="stage", bufs=2) as stage_pool:
            wp_sb = []
            for ecn in range(NEC):
                t = wp_pool.tile([128, D], BF16, tag="wp", name=f"wpt{ecn}")
                nc.sync.dma_start(t[:], wp[ecn * 128:(ecn + 1) * 128, :])
                wp_sb.append(t)
            for tcn in range(NTC):
                for ob in range(2):
                    ps_p = ps_tile("qkps", 2)
                    for ecn in range(NEC):
                        nc.tensor.matmul(
                            ps_p[:], ut_sb[ecn][:, tcn * 128:(tcn + 1) * 128],
                            wp_sb[ecn][:, ob * 512:(ob + 1) * 512],
                            start=(ecn == 0), stop=(ecn == NEC - 1))
                    st = stage_pool.tile([128, 512], F32, tag="st", name="stg")
                    nc.vector.tensor_copy(st[:], ps_p[:])
                    nc.sync.dma_start(
                        out[tcn * 128:(tcn + 1) * 128,
                            ob * 512:(ob + 1) * 512], st[:])


_NC_CACHE = None


def _get_nc():
    global _NC_CACHE
    if _NC_CACHE is None:
        _NC_CACHE = _build()
    return _NC_CACHE


def make_in_maps(embds, W_qkv, W_proj):
    embds = np.asarray(embds, dtype=np.float32)
    W_qkv = np.asarray(W_qkv, dtype=np.float32)
    W_proj = np.asarray(W_proj, dtype=np.float32)

    mask_np = np.triu(np.ones((128, 128))).astype(ml_dtypes.bfloat16)
    emat_np = np.kron(np.eye(HL), np.ones((1, Dh))).astype(np.float32)

    in_maps = []
    for c in range(NCORES):
        b, hg = c // 2, c % 2
        sl = slice(hg * DL, (hg + 1) * DL)
        in_maps.append({
            "xT": np.ascontiguousarray(embds[b].T),
            "wk": np.ascontiguousarray(W_qkv[:, 0 * D:1 * D][:, sl]),
            "wq": np.ascontiguousarray(W_qkv[:, 1 * D:2 * D][:, sl]),
            "wv": np.ascontiguousarray(W_qkv[:, 2 * D:3 * D][:, sl]),
            "wp": np.ascontiguousarray(W_proj[sl, :]).astype(ml_dtypes.bfloat16),
            "mask": mask_np,
            "emat": emat_np,
        })
    return in_maps


def gather_out(outs, b_proj):
    b_proj = np.asarray(b_proj, dtype=np.float32)
    full = np.empty((B, T, D), dtype=np.float32)
    for b in range(B):
        full[b] = outs[2 * b] + outs[2 * b + 1] + b_proj[None, :]
    return full


def kernel(embds, W_qkv, W_proj, b_proj):
    in_maps = make_in_maps(embds, W_qkv, W_proj)
    nc = _get_nc()
    res = run_bass_kernel_spmd(nc, in_maps, list(range(NCORES)))
    return gather_out([r["out"] for r in res.results], b_proj)


# revision 20
# speedup vs baseline: 1.1191x; 1.0297x over previous
"""Causal self-attention on 8 NeuronCores (Trainium2, Bass/Tile).

Sharding: core c handles batch b = c//2 and head-group hg = c%2
(8 of 16 heads = 512 of 1024 feature dims). W_qkv is split column-wise,
W_proj row-wise per head group; each core returns a partial [T, D]
projection output and the host sums the two partials per batch.

Per-core dataflow (all matmuls fp32r except P/V which are bf16):
  xT = embds[b].T              [1024, 2048]  (host-transposed)
  qT/kT = Wq/Wk.T @ x.T        [512, 2048]   (head-dim major)
  v     = x @ Wv               [2048, 512]   (natural, + ones col per head)
  sT[j,i] = kT.T @ qT          per head, causal-skipped/shrunk tiles
  PT = exp(SCALE * sT) (*mask on diagonal strips)   bf16
  UT[e,i], denom[i] = [v|1].T @ PT                  (ones col -> denom)
  affinT = UT * (1/denom)      broadcast via K=8 matmul with E matrix
  partial = affinT.T @ Wp      accumulated over e-chunks, DMA'd out
"""

import sys

for _p in ("/opt/trn_rl_repo",):
    if _p not in sys.path:
        sys.path.append(_p)

import ml_dtypes
import numpy as np

import concourse.bass as bass
import concourse.tile as tile
from concourse import bacc, mybir
from concourse.bass_utils import run_bass_kernel_spmd

F32 = mybir.dt.float32
F32R = mybir.dt.float32r
BF16 = mybir.dt.bfloat16
EXP = mybir.ActivationFunctionType.Exp
COPY = mybir.ActivationFunctionType.Copy

B, T, D = 4, 2048, 1024
H, Dh = 16, 64
SCALE = float(D) ** -0.5
NCORES = 8
DL = 512          # local (per-core) feature width = 8 heads * 64
HL = 8            # local heads
NDC = D // 128    # 8 d-chunks
NEC = DL // 128   # 4 e-chunks (head pairs)
NTB = T // 512    # 4 t-blocks of 512
NTC = T // 128    # 16 t-chunks of 128
VPAIR = 192       # v_sb per-pair block: [v_even(64) | one | junk(63) | v_odd(64)]
VROW = NEC * VPAIR  # 640 cols per v_sb tile


def _build():
    nc = bacc.Bacc("TRN2", target_bir_lowering=False, debug=False,
                   num_devices=NCORES)

    xT = nc.declare_dram_parameter("xT", [D, T], F32R, isOutput=False)
    wq = nc.declare_dram_parameter("wq", [D, DL], F32R, isOutput=False)
    wk = nc.declare_dram_parameter("wk", [D, DL], F32R, isOutput=False)
    wv = nc.declare_dram_parameter("wv", [D, DL], F32R, isOutput=False)
    wp = nc.declare_dram_parameter("wp", [DL, D], BF16, isOutput=False)
    mask = nc.declare_dram_parameter("mask", [128, 128], BF16, isOutput=False)
    emat = nc.declare_dram_parameter("emat", [HL, DL], F32, isOutput=False)
    out = nc.declare_dram_parameter("out", [T, D], F32, isOutput=True)

    with tile.TileContext(nc) as tc:
        _emit(nc, tc, xT, wq, wk, wv, wp, mask, emat, out)
    nc.compile()
    return nc


def _emit(nc, tc, xT, wq, wk, wv, wp, mask, emat, out):
    from contextlib import ExitStack

    ctx = ExitStack()
    with ctx:
        wqk_pool = ctx.enter_context(tc.tile_pool(name="wqk", bufs=16))
        qk_pool = ctx.enter_context(tc.tile_pool(name="qk", bufs=6))
        vsb_pool = ctx.enter_context(tc.tile_pool(name="vsb", bufs=NTC))
        ut_pool = ctx.enter_context(tc.tile_pool(name="ut", bufs=NEC))
        dn_pool = ctx.enter_context(tc.tile_pool(name="dn", bufs=1))
        dns_pool = ctx.enter_context(tc.tile_pool(name="dns", bufs=2))
        pt_pool = ctx.enter_context(tc.tile_pool(name="pt", bufs=3))
        cst_pool = ctx.enter_context(tc.tile_pool(name="cst", bufs=1))
        ps_pool = ctx.enter_context(tc.tile_pool(name="ps", bufs=1, space="PSUM"))

        # constants
        mk_sb = cst_pool.tile([128, 128], BF16, tag="mk")
        nc.sync.dma_start(mk_sb[:], mask[:])
        em_sb = cst_pool.tile([HL, DL], F32, tag="em")
        nc.sync.dma_start(em_sb[:], emat[:])

        # persistent tiles
        ut_sb = [ut_pool.tile([128, T], BF16, tag="ut", name=f"ut{i}")
                 for i in range(NEC)]
        dn_sb = dn_pool.tile([HL, T], F32, tag="dn")
        rd_sb = dn_pool.tile([HL, T], F32, tag="rd")
        v_sb = [vsb_pool.tile([128, VROW], BF16, tag="vsb", name=f"vsb{i}")
                for i in range(NTC)]
        # garbage rows of dn would hit reciprocal before they are written;
        # keep them finite so 0*inf NaNs can't leak out of the R matmul
        nc.gpsimd.memset(dn_sb[:], 1.0)

        def ps_tile(tag, bufs):
            return ps_pool.tile([128, 512], F32, tag=tag, name=f"ps_{tag}",
                                bufs=bufs)

        with tc.tile_pool(name="xt", bufs=NDC) as xt_pool:
            xt_sb = []
            qk_chunks = {}

            def qk_filler(ec):
                """Generator computing q/k chunks for `ec`; yields between
                small PE steps so it can be dripped into the attention loop
                as filler work that keeps the PE dense (HAM stays warm).
                kT is stored twice, zero-padded per head parity, so the
                score matmuls run with K=128 (K=64 fp32r matmuls throttle
                to half rate when ACT runs concurrently)."""
                wq_t, wk_t = [], []
                for dc in range(NDC):
                    t = wqk_pool.tile([128, 128], F32R, tag="wqk", name="wqkt")
                    nc.sync.dma_start(
                        t[:], wq[dc * 128:(dc + 1) * 128,
                                 ec * 128:(ec + 1) * 128])
                    wq_t.append(t)
                for dc in range(NDC):
                    t = wqk_pool.tile([128, 128], F32R, tag="wqk", name="wqkt")
                    nc.sync.dma_start(
                        t[:], wk[dc * 128:(dc + 1) * 128,
                                 ec * 128:(ec + 1) * 128])
                    wk_t.append(t)
                q_ec = qk_pool.tile([128, T], F32R, tag="qk", name="q_ec")
                kA = qk_pool.tile([128, T], F32R, tag="qk", name="kA")
                kB = qk_pool.tile([128, T], F32R, tag="qk", name="kB")
                nc.gpsimd.memset(kA[64:128, :].bitcast(F32), 0.0)
                nc.gpsimd.memset(kB[0:64, :].bitcast(F32), 0.0)
                qk_chunks[ec] = (q_ec, kA, kB)
                for (w_t, iskA) in ((wq_t, False), (wk_t, True)):
                    for tbp in range(2):
                        pss = [ps_tile("qkps", 2) for _ in range(2)]
                        for dc in range(NDC):
                            for i in range(2):
                                tb = 2 * tbp + i
                                nc.tensor.matmul(
                                    pss[i], w_t[dc][:],
                                    xt_sb[dc][:, tb * 512:(tb + 1) * 512],
                                    start=(dc == 0), stop=(dc == NDC - 1))
                            yield
                        for i in range(2):
                            tb = 2 * tbp + i
                            sl = slice(tb * 512, (tb + 1) * 512)
                            if iskA:
                                nc.vector.tensor_copy(
                                    kA[0:64, sl], pss[i][0:64, :])
                                nc.vector.tensor_copy(
                                    kB[64:128, sl], pss[i][64:128, :])
                            else:
                                nc.vector.tensor_copy(q_ec[:, sl], pss[i][:])
                        yield

            def normalize(ec):
                """affinT = UT * 1/denom for chunk ec (dripped into the next
                chunk's attention so nothing stalls on it)."""
                nc.vector.reciprocal_approx_fast(rd_sb[:], dn_sb[:])
                for tb in range(NTB):
                    ps_r = ps_tile("qkps", 2)
                    nc.tensor.matmul(
                        ps_r[:], em_sb[:, ec * 128:(ec + 1) * 128],
                        rd_sb[:, tb * 512:(tb + 1) * 512],
                        start=True, stop=True)
                    nc.vector.tensor_mul(
                        ut_sb[ec][:, tb * 512:(tb + 1) * 512],
                        ut_sb[ec][:, tb * 512:(tb + 1) * 512], ps_r[:])

            # ------------- phase A0: v = x @ Wv (+ dripped qk(0)) -------------
            with tc.tile_pool(name="wv", bufs=NDC) as wv_pool:
                wv_sb = []
                for dc in range(NDC):
                    t = wv_pool.tile([128, DL], F32R, tag="wv", name=f"wv{dc}")
                    nc.sync.dma_start(t[:], wv[dc * 128:(dc + 1) * 128, :])
                    wv_sb.append(t)
                    t = xt_pool.tile([128, T], F32R, tag="xt", name=f"xt{dc}")
                    nc.sync.dma_start(t[:], xT[dc * 128:(dc + 1) * 128, :])
                    xt_sb.append(t)

                filler0 = qk_filler(0)
                for tcn in range(NTC):
                    ps_v = ps_tile("utps", 2)
                    for dc in range(NDC):
                        nc.tensor.matmul(
                            ps_v[:], xt_sb[dc][:, tcn * 128:(tcn + 1) * 128],
                            wv_sb[dc][:], start=(dc == 0),
                            stop=(dc == NDC - 1))
                    dst = v_sb[tcn][:].rearrange("p (e c) -> p e c", c=VPAIR)
                    src = ps_v[:].rearrange("p (e c) -> p e c", c=128)
                    nc.vector.tensor_copy(dst[:, :, 0:64], src[:, :, 0:64])
                    nc.vector.tensor_copy(dst[:, :, 128:192], src[:, :, 64:128])
                    nc.gpsimd.memset(dst[:, :, 64:65], 1.0)
                    nc.gpsimd.memset(dst[:, :, 65:128], 0.0)
                    next(filler0, None)
                for _ in filler0:
                    pass
            # wv pool released here

            # ------------- per e-chunk: attention + dripped qk(ec+1) -------------
            for ec in range(NEC):
                filler = qk_filler(ec + 1) if ec + 1 < NEC else iter(())
                q_ec, kA, kB = qk_chunks.pop(ec)
                for par in range(2):       # head parity within chunk
                    h = 2 * ec + par       # local head index
                    kpad = kA if par == 0 else kB
                    for ibp in range(2):   # i-block pair (2*ibp, 2*ibp+1)
                        ibl, ibr = 2 * ibp, 2 * ibp + 1
                        utl = ps_tile("utps", 2)
                        utr = ps_tile("utps", 2)
                        for jt in range(4 * ibr + 4):
                            next(filler, None)
                            if ec > 0 and par == 0 and ibp == 0 and jt == 6:
                                normalize(ec - 1)
                            dl = (jt // 4 == ibl)
                            skip_l = (jt // 4 > ibl)
                            dr = (jt // 4 == ibr)
                            cl = 128 * (jt - 4 * ibl) if dl else 0
                            cr = 128 * (jt - 4 * ibr) if dr else 0
                            c0 = 512 + cr if skip_l else cl
                            st_ps = ps_pool.tile([128, 1024], F32, tag="stps",
                                                 name="ps_stps", bufs=2)
                            kh_j = kpad[:, jt * 128:(jt + 1) * 128]
                            if not skip_l:
                                nc.tensor.matmul(
                                    st_ps[:, cl:512], kh_j,
                                    q_ec[:, ibl * 512 + cl:(ibl + 1) * 512],
                                    start=True, stop=True)
                            nc.tensor.matmul(
                                st_ps[:, 512 + cr:1024], kh_j,
                                q_ec[:, ibr * 512 + cr:(ibr + 1) * 512],
                                start=True, stop=True)
                            pt_t = pt_pool.tile([128, 1024], BF16, tag="pt")
                            nc.scalar.activation(
                                pt_t[:, c0:1024], st_ps[:, c0:1024], EXP,
                                scale=SCALE)
                            if dl:
                                nc.vector.tensor_mul(
                                    pt_t[:, cl:cl + 128],
                                    pt_t[:, cl:cl + 128], mk_sb[:])
                            if dr:
                                nc.vector.tensor_mul(
                                    pt_t[:, 512 + cr:512 + cr + 128],
                                    pt_t[:, 512 + cr:512 + cr + 128], mk_sb[:])
                            # PV: [v|1].T @ PT -> UT rows + denom row
                            vt = v_sb[jt][:].rearrange(
                                "p (e c) -> p e c", c=VPAIR)[:, ec, :]
                            if par == 0:
                                lhs = vt[:, 0:65]       # M=65 -> rows 0..64
                                rsl = slice(0, 65)
                            else:
                                # [one|junk63|v_odd]: denom row 0, v 64..127
                                lhs = vt[:, 64:192]     # M=128
                                rsl = slice(0, 128)
                            if not skip_l:
                                nc.tensor.matmul(
                                    utl[rsl, cl:512], lhs, pt_t[:, cl:512],
                                    start=(jt == 0), stop=(jt == 4 * ibl + 3),
                                    skip_group_check=True)
                            nc.tensor.matmul(
                                utr[rsl, cr:512], lhs, pt_t[:, 512 + cr:1024],
                                start=(jt == 0), stop=(jt == 4 * ibr + 3),
                                skip_group_check=True)
                            for ib_d, ut_d in ((ibl, utl), (ibr, utr)):
                                if jt != 4 * ib_d + 3:
                                    continue
                                if par == 0:
                                    usrc, dsrc, r = (ut_d[0:64, :],
                                                     ut_d[64:65, :], 64)
                                    udst = ut_sb[ec][
                                        0:64, ib_d * 512:(ib_d + 1) * 512]
                                else:
                                    usrc, dsrc, r = (ut_d[64:128, :],
                                                     ut_d[0:1, :], 0)
                                    udst = ut_sb[ec][
                                        64:128, ib_d * 512:(ib_d + 1) * 512]
                                nc.vector.tensor_copy(udst, usrc)
                                # denom: same-partition copy, then DMA repack
                                stg = dns_pool.tile([128, 512], F32,
                                                    tag="dns", name="dnstg")
                                nc.vector.tensor_copy(stg[r:r + 1, :], dsrc)
                                nc.sync.dma_start(
                                    dn_sb[h:h + 1,
                                          ib_d * 512:(ib_d + 1) * 512],
                                    stg[r:r + 1, :])
                for _ in filler:   # drain remaining qk(ec+1) work
                    pass
            normalize(NEC - 1)
        # xt pool released here

        # ---------------- projection: out = affinT.T @ wp (bf16) ----------------
        with tc.tile_pool(name="wp", bufs=NEC) as wp_pool, \
             tc.tile_pool(name="stage", bufs=2) as stage_pool:
            wp_sb = []
            for ecn in range(NEC):
                t = wp_pool.tile([128, D], BF16, tag="wp", name=f"wpt{ecn}")
                nc.sync.dma_start(t[:], wp[ecn * 128:(ecn + 1) * 128, :])
                wp_sb.append(t)
            for tcn in range(NTC):
                for ob in range(2):
                    ps_p = ps_tile("qkps", 2)
                    for ecn in range(NEC):
                        nc.tensor.matmul(
                            ps_p[:], ut_sb[ecn][:, tcn * 128:(tcn + 1) * 128],
                            wp_sb[ecn][:, ob * 512:(ob + 1) * 512],
                            start=(ecn == 0), stop=(ecn == NEC - 1))
                    st = stage_pool.tile([128, 512], F32, tag="st", name="stg")
                    nc.vector.tensor_copy(st[:], ps_p[:])
                    nc.sync.dma_start(
                        out[tcn * 128:(tcn + 1) * 128,
                            ob * 512:(ob + 1) * 512], st[:])


_NC_CACHE = None


def _get_nc():
    global _NC_CACHE
    if _NC_CACHE is None:
        _NC_CACHE = _build()
    return _NC_CACHE


def make_in_maps(embds, W_qkv, W_proj):
    embds = np.asarray(embds, dtype=np.float32)
    W_qkv = np.asarray(W_qkv, dtype=np.float32)
    W_proj = np.asarray(W_proj, dtype=np.float32)

    mask_np = np.triu(np.ones((128, 128))).astype(ml_dtypes.bfloat16)
    emat_np = np.kron(np.eye(HL), np.ones((1, Dh))).astype(np.float32)

    in_maps = []
    for c in range(NCORES):
        b, hg = c // 2, c % 2
        sl = slice(hg * DL, (hg + 1) * DL)
        in_maps.append({
            "xT": np.ascontiguousarray(embds[b].T),
            "wk": np.ascontiguousarray(W_qkv[:, 0 * D:1 * D][:, sl]),
            "wq": np.ascontiguousarray(W_qkv[:, 1 * D:2 * D][:, sl]),
            "wv": np.ascontiguousarray(W_qkv[:, 2 * D:3 * D][:, sl]),
            "wp": np.ascontiguousarray(W_proj[sl, :]).astype(ml_dtypes.bfloat16),
            "mask": mask_np,
            "emat": emat_np,
        })
    return in_maps


def gather_out(outs, b_proj):
    b_proj = np.asarray(b_proj, dtype=np.float32)
    full = np.empty((B, T, D), dtype=np.float32)
    for b in range(B):
        full[b] = outs[2 * b] + outs[2 * b + 1] + b_proj[None, :]
    return full


def kernel(embds, W_qkv, W_proj, b_proj):
    in_maps = make_in_maps(embds, W_qkv, W_proj)
    nc = _get_nc()
    res = run_bass_kernel_spmd(nc, in_maps, list(range(NCORES)))
    return gather_out([r["out"] for r in res.results], b_proj)


# revision 21
# speedup vs baseline: 1.1328x; 1.0123x over previous
"""Causal self-attention on 8 NeuronCores (Trainium2, Bass/Tile).

Sharding: core c handles batch b = c//2 and head-group hg = c%2
(8 of 16 heads = 512 of 1024 feature dims). W_qkv is split column-wise,
W_proj row-wise per head group; each core returns a partial [T, D]
projection output and the host sums the two partials per batch.

Per-core dataflow (all matmuls fp32r except P/V which are bf16):
  xT = embds[b].T              [1024, 2048]  (host-transposed)
  qT/kT = Wq/Wk.T @ x.T        [512, 2048]   (head-dim major)
  v     = x @ Wv               [2048, 512]   (natural, + ones col per head)
  sT[j,i] = kT.T @ qT          per head, causal-skipped/shrunk tiles
  PT = exp(SCALE * sT) (*mask on diagonal strips)   bf16
  UT[e,i], denom[i] = [v|1].T @ PT                  (ones col -> denom)
  affinT = UT * (1/denom)      broadcast via K=8 matmul with E matrix
  partial = affinT.T @ Wp      accumulated over e-chunks, DMA'd out
"""

import sys

for _p in ("/opt/trn_rl_repo",):
    if _p not in sys.path:
        sys.path.append(_p)

import ml_dtypes
import numpy as np

import concourse.bass as bass
import concourse.tile as tile
from concourse import bacc, mybir
from concourse.bass_utils import run_bass_kernel_spmd

F32 = mybir.dt.float32
F32R = mybir.dt.float32r
BF16 = mybir.dt.bfloat16
EXP = mybir.ActivationFunctionType.Exp
COPY = mybir.ActivationFunctionType.Copy

B, T, D = 4, 2048, 1024
H, Dh = 16, 64
SCALE = float(D) ** -0.5
NCORES = 8
DL = 512          # local (per-core) feature width = 8 heads * 64
HL = 8            # local heads
NDC = D // 128    # 8 d-chunks
NEC = DL // 128   # 4 e-chunks (head pairs)
NTB = T // 512    # 4 t-blocks of 512
NTC = T // 128    # 16 t-chunks of 128
VPAIR = 192       # v_sb per-pair block: [v_even(64) | one | junk(63) | v_odd(64)]
VROW = NEC * VPAIR  # 640 cols per v_sb tile


def _build():
    nc = bacc.Bacc("TRN2", target_bir_lowering=False, debug=False,
                   num_devices=NCORES)

    xT = nc.declare_dram_parameter("xT", [D, T], F32R, isOutput=False)
    wq = nc.declare_dram_parameter("wq", [D, DL], F32R, isOutput=False)
    wk = nc.declare_dram_parameter("wk", [D, DL], F32R, isOutput=False)
    wv = nc.declare_dram_parameter("wv", [D, DL], F32R, isOutput=False)
    wp = nc.declare_dram_parameter("wp", [DL, D], BF16, isOutput=False)
    mask = nc.declare_dram_parameter("mask", [128, 128], BF16, isOutput=False)
    emat = nc.declare_dram_parameter("emat", [HL, DL], F32, isOutput=False)
    outA = nc.declare_dram_parameter("outA", [T, D], F32, isOutput=True)
    outB = nc.declare_dram_parameter("outB", [T, D], F32, isOutput=True)

    with tile.TileContext(nc) as tc:
        _emit(nc, tc, xT, wq, wk, wv, wp, mask, emat, outA, outB)
    nc.compile()
    return nc


def _emit(nc, tc, xT, wq, wk, wv, wp, mask, emat, outA, outB):
    from contextlib import ExitStack

    ctx = ExitStack()
    with ctx:
        wqk_pool = ctx.enter_context(tc.tile_pool(name="wqk", bufs=16))
        qk_pool = ctx.enter_context(tc.tile_pool(name="qk", bufs=6))
        vsb_pool = ctx.enter_context(tc.tile_pool(name="vsb", bufs=NTC))
        ut_pool = ctx.enter_context(tc.tile_pool(name="ut", bufs=NEC))
        dn_pool = ctx.enter_context(tc.tile_pool(name="dn", bufs=1))
        dns_pool = ctx.enter_context(tc.tile_pool(name="dns", bufs=2))
        pt_pool = ctx.enter_context(tc.tile_pool(name="pt", bufs=3))
        cst_pool = ctx.enter_context(tc.tile_pool(name="cst", bufs=1))
        ps_pool = ctx.enter_context(tc.tile_pool(name="ps", bufs=1, space="PSUM"))

        # constants
        mk_sb = cst_pool.tile([128, 128], BF16, tag="mk")
        nc.sync.dma_start(mk_sb[:], mask[:])
        em_sb = cst_pool.tile([HL, DL], F32, tag="em")
        nc.sync.dma_start(em_sb[:], emat[:])

        # persistent tiles
        ut_sb = [ut_pool.tile([128, T], BF16, tag="ut", name=f"ut{i}")
                 for i in range(NEC)]
        dn_sb = dn_pool.tile([HL, T], F32, tag="dn")
        rd_sb = dn_pool.tile([HL, T], F32, tag="rd")
        v_sb = [vsb_pool.tile([128, VROW], BF16, tag="vsb", name=f"vsb{i}")
                for i in range(NTC)]
        # garbage rows of dn would hit reciprocal before they are written;
        # keep them finite so 0*inf NaNs can't leak out of the R matmul
        nc.gpsimd.memset(dn_sb[:], 1.0)

        def ps_tile(tag, bufs):
            return ps_pool.tile([128, 512], F32, tag=tag, name=f"ps_{tag}",
                                bufs=bufs)

        with tc.tile_pool(name="xt", bufs=NDC) as xt_pool:
            xt_sb = []
            qk_chunks = {}

            def qk_filler(ec):
                """Generator computing q/k chunks for `ec`; yields between
                small PE steps so it can be dripped into the attention loop
                as filler work that keeps the PE dense (HAM stays warm).
                kT is stored twice, zero-padded per head parity, so the
                score matmuls run with K=128 (K=64 fp32r matmuls throttle
                to half rate when ACT runs concurrently)."""
                wq_t, wk_t = [], []
                for dc in range(NDC):
                    t = wqk_pool.tile([128, 128], F32R, tag="wqk", name="wqkt")
                    nc.sync.dma_start(
                        t[:], wq[dc * 128:(dc + 1) * 128,
                                 ec * 128:(ec + 1) * 128])
                    wq_t.append(t)
                for dc in range(NDC):
                    t = wqk_pool.tile([128, 128], F32R, tag="wqk", name="wqkt")
                    nc.sync.dma_start(
                        t[:], wk[dc * 128:(dc + 1) * 128,
                                 ec * 128:(ec + 1) * 128])
                    wk_t.append(t)
                q_ec = qk_pool.tile([128, T], F32R, tag="qk", name="q_ec")
                kA = qk_pool.tile([128, T], F32R, tag="qk", name="kA")
                kB = qk_pool.tile([128, T], F32R, tag="qk", name="kB")
                nc.gpsimd.memset(kA[64:128, :].bitcast(F32), 0.0)
                nc.gpsimd.memset(kB[0:64, :].bitcast(F32), 0.0)
                qk_chunks[ec] = (q_ec, kA, kB)
                for (w_t, iskA) in ((wq_t, False), (wk_t, True)):
                    for tbp in range(2):
                        pss = [ps_tile("qkps", 2) for _ in range(2)]
                        for dc in range(NDC):
                            for i in range(2):
                                tb = 2 * tbp + i
                                nc.tensor.matmul(
                                    pss[i], w_t[dc][:],
                                    xt_sb[dc][:, tb * 512:(tb + 1) * 512],
                                    start=(dc == 0), stop=(dc == NDC - 1))
                            yield
                        for i in range(2):
                            tb = 2 * tbp + i
                            sl = slice(tb * 512, (tb + 1) * 512)
                            if iskA:
                                nc.vector.tensor_copy(
                                    kA[0:64, sl], pss[i][0:64, :])
                                nc.vector.tensor_copy(
                                    kB[64:128, sl], pss[i][64:128, :])
                            else:
                                nc.vector.tensor_copy(q_ec[:, sl], pss[i][:])
                        yield

            def normalize(ec):
                """affinT = UT * 1/denom for chunk ec (dripped into the next
                chunk's attention so nothing stalls on it)."""
                nc.vector.reciprocal_approx_fast(rd_sb[:], dn_sb[:])
                for tb in range(NTB):
                    ps_r = ps_tile("qkps", 2)
                    nc.tensor.matmul(
                        ps_r[:], em_sb[:, ec * 128:(ec + 1) * 128],
                        rd_sb[:, tb * 512:(tb + 1) * 512],
                        start=True, stop=True)
                    nc.vector.tensor_mul(
                        ut_sb[ec][:, tb * 512:(tb + 1) * 512],
                        ut_sb[ec][:, tb * 512:(tb + 1) * 512], ps_r[:])

            # ------------- phase A0: v = x @ Wv (+ dripped qk(0)) -------------
            with tc.tile_pool(name="wv", bufs=NDC) as wv_pool:
                wv_sb = []
                for dc in range(NDC):
                    t = wv_pool.tile([128, DL], F32R, tag="wv", name=f"wv{dc}")
                    nc.sync.dma_start(t[:], wv[dc * 128:(dc + 1) * 128, :])
                    wv_sb.append(t)
                    t = xt_pool.tile([128, T], F32R, tag="xt", name=f"xt{dc}")
                    nc.sync.dma_start(t[:], xT[dc * 128:(dc + 1) * 128, :])
                    xt_sb.append(t)

                filler0 = qk_filler(0)
                for tcn in range(NTC):
                    ps_v = ps_tile("utps", 2)
                    for dc in range(NDC):
                        nc.tensor.matmul(
                            ps_v[:], xt_sb[dc][:, tcn * 128:(tcn + 1) * 128],
                            wv_sb[dc][:], start=(dc == 0),
                            stop=(dc == NDC - 1))
                    dst = v_sb[tcn][:].rearrange("p (e c) -> p e c", c=VPAIR)
                    src = ps_v[:].rearrange("p (e c) -> p e c", c=128)
                    nc.vector.tensor_copy(dst[:, :, 0:64], src[:, :, 0:64])
                    nc.vector.tensor_copy(dst[:, :, 128:192], src[:, :, 64:128])
                    nc.gpsimd.memset(dst[:, :, 64:65], 1.0)
                    nc.gpsimd.memset(dst[:, :, 65:128], 0.0)
                    next(filler0, None)
                for _ in filler0:
                    pass
            # wv pool released here

            # ------------- per e-chunk: attention + dripped filler work -------------
            def attention_chunk(ec, drip):
                q_ec, kA, kB = qk_chunks.pop(ec)
                slot = [0]
                for par in range(2):       # head parity within chunk
                    h = 2 * ec + par       # local head index
                    kpad = kA if par == 0 else kB
                    for ibp in range(2):   # i-block pair (2*ibp, 2*ibp+1)
                        ibl, ibr = 2 * ibp, 2 * ibp + 1
                        utl = ps_tile("utps", 2)
                        utr = ps_tile("utps", 2)
                        for jt in range(4 * ibr + 4):
                            drip(slot[0])
                            slot[0] += 1
                            dl = (jt // 4 == ibl)
                            skip_l = (jt // 4 > ibl)
                            dr = (jt // 4 == ibr)
                            cl = 128 * (jt - 4 * ibl) if dl else 0
                            cr = 128 * (jt - 4 * ibr) if dr else 0
                            c0 = 512 + cr if skip_l else cl
                            st_ps = ps_pool.tile([128, 1024], F32, tag="stps",
                                                 name="ps_stps", bufs=2)
                            kh_j = kpad[:, jt * 128:(jt + 1) * 128]
                            if not skip_l:
                                nc.tensor.matmul(
                                    st_ps[:, cl:512], kh_j,
                                    q_ec[:, ibl * 512 + cl:(ibl + 1) * 512],
                                    start=True, stop=True)
                            nc.tensor.matmul(
                                st_ps[:, 512 + cr:1024], kh_j,
                                q_ec[:, ibr * 512 + cr:(ibr + 1) * 512],
                                start=True, stop=True)
                            pt_t = pt_pool.tile([128, 1024], BF16, tag="pt")
                            nc.scalar.activation(
                                pt_t[:, c0:1024], st_ps[:, c0:1024], EXP,
                                scale=SCALE)
                            if dl:
                                nc.vector.tensor_mul(
                                    pt_t[:, cl:cl + 128],
                                    pt_t[:, cl:cl + 128], mk_sb[:])
                            if dr:
                                nc.vector.tensor_mul(
                                    pt_t[:, 512 + cr:512 + cr + 128],
                                    pt_t[:, 512 + cr:512 + cr + 128], mk_sb[:])
                            # PV: [v|1].T @ PT -> UT rows + denom row
                            vt = v_sb[jt][:].rearrange(
                                "p (e c) -> p e c", c=VPAIR)[:, ec, :]
                            if par == 0:
                                lhs = vt[:, 0:65]       # M=65 -> rows 0..64
                                rsl = slice(0, 65)
                            else:
                                # [one|junk63|v_odd]: denom row 0, v 64..127
                                lhs = vt[:, 64:192]     # M=128
                                rsl = slice(0, 128)
                            if not skip_l:
                                nc.tensor.matmul(
                                    utl[rsl, cl:512], lhs, pt_t[:, cl:512],
                                    start=(jt == 0), stop=(jt == 4 * ibl + 3),
                                    skip_group_check=True)
                            nc.tensor.matmul(
                                utr[rsl, cr:512], lhs, pt_t[:, 512 + cr:1024],
                                start=(jt == 0), stop=(jt == 4 * ibr + 3),
                                skip_group_check=True)
                            for ib_d, ut_d in ((ibl, utl), (ibr, utr)):
                                if jt != 4 * ib_d + 3:
                                    continue
                                if par == 0:
                                    usrc, dsrc, r = (ut_d[0:64, :],
                                                     ut_d[64:65, :], 64)
                                    udst = ut_sb[ec][
                                        0:64, ib_d * 512:(ib_d + 1) * 512]
                                else:
                                    usrc, dsrc, r = (ut_d[64:128, :],
                                                     ut_d[0:1, :], 0)
                                    udst = ut_sb[ec][
                                        64:128, ib_d * 512:(ib_d + 1) * 512]
                                nc.vector.tensor_copy(udst, usrc)
                                # denom: same-partition copy, then DMA repack
                                stg = dns_pool.tile([128, 512], F32,
                                                    tag="dns", name="dnstg")
                                nc.vector.tensor_copy(stg[r:r + 1, :], dsrc)
                                nc.sync.dma_start(
                                    dn_sb[h:h + 1,
                                          ib_d * 512:(ib_d + 1) * 512],
                                    stg[r:r + 1, :])

            for ec in range(NEC - 1):
                filler = qk_filler(ec + 1)

                def drip(slot, ec=ec, filler=filler):
                    next(filler, None)
                    if ec > 0 and slot == 6:
                        normalize(ec - 1)

                attention_chunk(ec, drip)
                for _ in filler:   # drain remaining qk(ec+1) work
                    pass
        # xt pool released here (before the last attention chunk)

        with tc.tile_pool(name="wp", bufs=NEC) as wp_pool, \
             tc.tile_pool(name="stage", bufs=3) as stage_pool:
            wp_sb = []
            for ecn in range(NEC):
                t = wp_pool.tile([128, D], BF16, tag="wp", name=f"wpt{ecn}")
                nc.sync.dma_start(t[:], wp[ecn * 128:(ecn + 1) * 128, :])
                wp_sb.append(t)

            def proj_pass(ecs, out_t):
                """One projection pass accumulating a subset of e-chunks
                into its own partial output (summed on the host)."""
                for tcn in range(NTC):
                    for ob in range(2):
                        ps_p = ps_tile("qkps", 2)
                        for i, ecn in enumerate(ecs):
                            nc.tensor.matmul(
                                ps_p[:],
                                ut_sb[ecn][:, tcn * 128:(tcn + 1) * 128],
                                wp_sb[ecn][:, ob * 512:(ob + 1) * 512],
                                start=(i == 0), stop=(i == len(ecs) - 1))
                        st = stage_pool.tile([128, 512], F32, tag="st",
                                             name="stg")
                        nc.vector.tensor_copy(st[:], ps_p[:])
                        nc.sync.dma_start(
                            out_t[tcn * 128:(tcn + 1) * 128,
                                  ob * 512:(ob + 1) * 512], st[:])
                        yield

            # last attention chunk: drip proj pass A (chunks 0-1) as filler
            projA = proj_pass((0, 1), outA)

            def drip3(slot):
                if slot == 6:
                    normalize(NEC - 2)
                if slot >= 8:
                    next(projA, None)

            attention_chunk(NEC - 1, drip3)
            for _ in projA:
                pass
            normalize(NEC - 1)
            for _ in proj_pass((2, 3), outB):
                pass



_NC_CACHE = None


def _get_nc():
    global _NC_CACHE
    if _NC_CACHE is None:
        _NC_CACHE = _build()
    return _NC_CACHE


def make_in_maps(embds, W_qkv, W_proj):
    embds = np.asarray(embds, dtype=np.float32)
    W_qkv = np.asarray(W_qkv, dtype=np.float32)
    W_proj = np.asarray(W_proj, dtype=np.float32)

    mask_np = np.triu(np.ones((128, 128))).astype(ml_dtypes.bfloat16)
    emat_np = np.kron(np.eye(HL), np.ones((1, Dh))).astype(np.float32)

    in_maps = []
    for c in range(NCORES):
        b, hg = c // 2, c % 2
        sl = slice(hg * DL, (hg + 1) * DL)
        in_maps.append({
            "xT": np.ascontiguousarray(embds[b].T),
            "wk": np.ascontiguousarray(W_qkv[:, 0 * D:1 * D][:, sl]),
            "wq": np.ascontiguousarray(W_qkv[:, 1 * D:2 * D][:, sl]),
            "wv": np.ascontiguousarray(W_qkv[:, 2 * D:3 * D][:, sl]),
            "wp": np.ascontiguousarray(W_proj[sl, :]).astype(ml_dtypes.bfloat16),
            "mask": mask_np,
            "emat": emat_np,
        })
    return in_maps


def gather_out(outs, b_proj):
    b_proj = np.asarray(b_proj, dtype=np.float32)
    full = np.empty((B, T, D), dtype=np.float32)
    for b in range(B):
        full[b] = outs[2 * b] + outs[2 * b + 1] + b_proj[None, :]
    return full


def kernel(embds, W_qkv, W_proj, b_proj):
    in_maps = make_in_maps(embds, W_qkv, W_proj)
    nc = _get_nc()
    res = run_bass_kernel_spmd(nc, in_maps, list(range(NCORES)))
    return gather_out([r["outA"] + r["outB"] for r in res.results], b_proj)
